# revision 20
# baseline (speedup 1.0000x reference)
"""Trainium2 Bass kernel for nn_Attention_org_single_85074712199391.

Channel-attention module. Reference math (per batch b, head h):
    Qc = emb1[b].reshape(N, 4, dq)[:, h]          # [N, 128]
    Kc = emb_all[b].reshape(N, 4, dk)[:, h]       # [N, 240]
    Q = Qc @ Wq[h].T ; K = Kc @ Wk.T ; V = Kc @ Wv.T
    scores = Q.T @ K / sqrt(KV)                   # [128, 240]
    probs = softmax(instnorm(scores), axis=-1)
    context = probs @ V.T                         # [128, N]
    O1 = permute/concat(context) @ Wo.T           # [N, 512]

Algebraic rewrite used here (exact):
    S_h      = Qc.T @ Kc                          # big contraction over N
    scores_h = (Wq[h]/sqrt(KV)) @ S_h @ Wk.T
    probs_h  = softmax over dk of rstd*scores_h   # mean cancels in softmax
    P2_h     = probs_h @ Wv                       # [128, 240]
    ctx_h    = P2_h @ Kc.T                        # [128, N]
    O1       = sum_h ctx_h.T @ Wo[:, h::4].T     # accumulate over heads

Phases per core (core b owns batch b; weights replicated; no collectives):
    A: stream e1/ea as bf16 (DMA-cast loads on the gpsimd SWDGE queue);
       accumulate S_h in PSUM; transpose ea into a resident bf16 eaT via
       XBAR DMA transposes on the two HWDGE queues (SP + ACT) so the PE
       only does the S matmuls. dk=240 per head is covered by two
       overlapping 128-wide source slices [0:128) and [112:240) (XBAR
       free dim must be a multiple of 128); phase C contracts 112+128
       partitions accordingly.
    B: tiny fp32 scores path, instance-norm stats via ones-matmul
       (partition reduce + broadcast in one op), batched softmax across
       heads; the softmax denominator is folded into e_all right after
       exp, so downstream context needs no per-row rescale. exp.T via
       XBAR DMA transpose as well; P2.T in bf16.
    C: context matmuls off eaT, then the output projection accumulating
       over heads; PSUM evacuations are plain copies spread round-robin
       over DVE / ACT / Pool so no single engine stalls the PE.
"""

import sys

import numpy as np

try:
    import concourse.bass as bass
except ImportError:  # harness environments without the repo on sys.path
    sys.path.insert(0, "/opt/trn_rl_repo")
    import concourse.bass as bass

import concourse.bacc as bacc

import ml_dtypes
import concourse.mybir as mybir
import concourse.tile as tile
from concourse.bass_utils import run_bass_kernel_spmd

F32 = mybir.dt.float32
BF16 = mybir.dt.bfloat16
AF = mybir.ActivationFunctionType
ALU = mybir.AluOpType

B, N, C, KV, H = 8, 4096, 512, 960, 4
DQ, DK = C // 4, KV // 4          # 128, 240
PT = 128                          # partition tile
NT = N // PT                      # 32 row tiles
NCH = N // 512                    # 8 column chunks for phase C
KCH = 2                           # dk split for 240-deep contractions
KHW = DK // KCH                   # 120 (scores path only)
OV0, OV1 = 112, 128               # overlapping dk chunks: [0:128) -> use 112,
                                  # [112:240) -> use 128
EPS = 1e-5
NORM_CNT = float(DQ * DK)         # instance-norm element count

import os as _os
PA_BUFS = int(_os.environ.get("PA_BUFS", "4"))
PSB_BUFS = int(_os.environ.get("PSB_BUFS", "4"))
CX_BUFS = int(_os.environ.get("CX_BUFS", "3"))
O_BUFS = int(_os.environ.get("O_BUFS", "3"))


def build_nc(ablate=frozenset(), reps=1, serialize=False, dbg=False):
    nc = bacc.Bacc("TRN2", target_bir_lowering=False, debug=False)

    e1 = nc.dram_tensor("e1", [N, C], F32, kind="ExternalInput").ap()
    ea = nc.dram_tensor("ea", [N, KV], F32, kind="ExternalInput").ap()
    wqt = nc.dram_tensor("wqt", [DQ, H, DQ], F32, kind="ExternalInput").ap()
    wkt = nc.dram_tensor("wkt", [KHW, KCH, DK], F32, kind="ExternalInput").ap()
    wvb = nc.dram_tensor("wvb", [PT, KCH, DK], BF16, kind="ExternalInput").ap()
    wotb = nc.dram_tensor("wotb", [DQ, H, C], BF16, kind="ExternalInput").ap()
    o1 = nc.dram_tensor("o1", [N, C], F32, kind="ExternalOutput").ap()
    dbg_t = None
    if dbg:
        dbg_t = {
            "d_s": nc.dram_tensor("d_s", [DQ, H, DK], F32,
                                  kind="ExternalOutput").ap(),
            "d_e": nc.dram_tensor("d_e", [DQ, H, DK], BF16,
                                  kind="ExternalOutput").ap(),
            "d_p2t": nc.dram_tensor("d_p2t", [PT, H, 2, DQ], BF16,
                                    kind="ExternalOutput").ap(),
            "d_eaT": nc.dram_tensor("d_eaT", [PT, 2 * H, N], BF16,
                                    kind="ExternalOutput").ap(),
        }

    with tile.TileContext(nc) as tc:
        for r in range(reps):
            if serialize and r:
                tc.strict_bb_all_engine_barrier()
            build_body(tc, e1, ea, wqt, wkt, wvb, wotb, o1, ablate, dbg_t)
    nc.compile()
    return nc


def build_body(tc, e1, ea, wqt, wkt, wvb, wotb, o1, ablate=frozenset(),
               dbg_t=None):
    nc = tc.nc
    from contextlib import ExitStack, nullcontext

    with ExitStack() as stk:
        pW = stk.enter_context(tc.tile_pool(name="persist", bufs=1))

        # --- persistent weights / constants -----------------------------------
        wqt_sb = pW.tile([DQ, H, DQ], F32, tag="wqt_sb")
        nc.scalar.dma_start(wqt_sb[:], wqt[:])
        wkt_sb = pW.tile([KHW, KCH, DK], F32, tag="wkt_sb")
        nc.scalar.dma_start(wkt_sb[:], wkt[:])
        wvb_sb = pW.tile([PT, KCH, DK], BF16, tag="wvb_sb")
        nc.scalar.dma_start(wvb_sb[:], wvb[:])
        wotb_sb = pW.tile([DQ, H, C], BF16, tag="wotb_sb")
        nc.scalar.dma_start(wotb_sb[:], wotb[:])
        ones_sb = pW.tile([PT, PT], F32, tag="ones_sb")
        nc.vector.memset(ones_sb[:], 1.0)
        eps_sb = pW.tile([PT, 1], F32, tag="eps_sb")
        nc.vector.memset(eps_sb[:], EPS)

        # --- persistent activations -------------------------------------------
        # eaT chunk 2h+c: c=0 holds k in [240h, 240h+128) (112 used),
        #                 c=1 holds k in [240h+112, 240h+240)
        eaT_sb = pW.tile([PT, 2 * H, N], BF16, tag="eaT_sb")
        s_sb = pW.tile([DQ, H, DK], F32, tag="s_sb")           # S_h
        sc_all = pW.tile([DQ, H, DK], F32, tag="sc_all")       # scores
        e_all = pW.tile([DQ, H, DK], BF16, tag="e_all")        # exp()
        stats = pW.tile([DQ, H, 2], F32, tag="stats")          # row sums, sq
        ptT_sb = pW.tile([PT, H, KCH, DQ], BF16, tag="ptT_sb")  # exp.T chunks
        p2t_sb = pW.tile([PT, H, 2, DQ], BF16, tag="p2t_sb")   # P2.T chunks

        def small(name):
            return pW.tile([DQ, H], F32, tag=name, name=name)

        mu_all = small("mu_all")
        m2_all = small("m2_all")
        mu2_all = small("mu2_all")
        var_all = small("var_all")
        sd_all = small("sd_all")
        rstd_all = small("rstd_all")
        den_all = small("den_all")
        rec_all = small("rec_all")

        pA = stk.enter_context(tc.tile_pool(name="pA", bufs=PA_BUFS))
        pAe = stk.enter_context(
            tc.tile_pool(name="pAe", bufs=int(_os.environ.get("PAE_BUFS", "4"))))
        pBs = stk.enter_context(tc.tile_pool(name="pBs", bufs=2))
        pC = stk.enter_context(tc.tile_pool(name="pC", bufs=4))

        # --- phase A: bf16 DMA-cast loads; S accumulation; eaT via XBAR -------
        with tc.tile_pool(name="psS", bufs=1, space="PSUM") as psS:
            s_ps = [psS.tile([DQ, DK], F32, tag=f"s{h}", name=f"s_ps{h}")
                    for h in range(H)]
            for ii in range(NT // 4):
                # four 128-row tiles per DMA: row a*128+p of the group lands
                # on partition p, free-slot a (fewer, larger DMA transfers)
                prows = slice(ii * 4 * PT, (ii + 1) * 4 * PT)
                e1b = pA.tile([PT, 4, C], BF16, tag="e1b")
                nc.gpsimd.dma_start(
                    e1b[:], e1[prows, :].rearrange("(a p) k -> p a k", p=PT))
                eab = pAe.tile([PT, 4, KV], BF16, tag="eab")
                nc.gpsimd.dma_start(
                    eab[:], ea[prows, :].rearrange("(a p) k -> p a k", p=PT))
                for a in range(4):
                    i = 4 * ii + a
                    rows = slice(i * PT, (i + 1) * PT)
                    if "sA" not in ablate:
                        for h in range(H):
                            nc.tensor.matmul(
                                s_ps[h][:],
                                e1b[:, a, h * DQ:(h + 1) * DQ],
                                eab[:, a, h * DK:(h + 1) * DK],
                                start=(i == 0),
                                stop=(i == NT - 1),
                            )
                    if "tpose" not in ablate:
                        # ALL XBAR transposes go on the SP queue: concurrent
                        # XBAR transposes on two HWDGE queues corrupt each
                        # other (observed on hw: wrong data in ~1/5 slabs).
                        for h in range(H):
                            for c in range(2):
                                k0 = h * DK + (0 if c == 0 else DK - 128)
                                nc.sync.dma_start(
                                    eaT_sb[:, 2 * h + c, rows],
                                    eab[:, a, k0:k0 + 128],
                                    transpose=True)
            if "sA" not in ablate:
                for h in range(H):
                    nc.scalar.copy(s_sb[:, h, :], s_ps[h][:])

        # --- phase B -----------------------------------------------------------
        with (tc.tile_pool(name="psB", bufs=PSB_BUFS, space="PSUM")
              if "phaseB" not in ablate else nullcontext()) as psB:
            for h in range(H if "phaseB" not in ablate else 0):
                # U.T = (S.T-chunks) @ (Wq_h.T/sqrt(KV))  [240k, 128e], fp32
                # (computed directly in transposed form: S as the stationary
                # operand; avoids a PE transpose round-trip through PSUM)
                ut_sb = pBs.tile([KHW, KCH, DQ], F32, tag="ut_sb", name="ut_sb")
                for j in range(KCH):
                    ut_ps = psB.tile([KHW, DQ], F32, tag="psb", name="ut_ps")
                    nc.tensor.matmul(ut_ps[:],
                                     s_sb[:, h, j * KHW:(j + 1) * KHW],
                                     wqt_sb[:, h, :], start=True, stop=True)
                    nc.vector.tensor_copy(ut_sb[:, j, :], ut_ps[:])
                # scores = U @ Wk.T  [128e, 240ek], fp32
                sc_ps = psB.tile([DQ, DK], F32, tag="psb", name="sc_ps")
                for j in range(KCH):
                    nc.tensor.matmul(sc_ps[:], ut_sb[:, j, :], wkt_sb[:, j, :],
                                     start=(j == 0), stop=(j == KCH - 1))
                # evacuate + per-row sums of x and x^2 for instance-norm
                nc.scalar.activation(sc_all[:, h, :], sc_ps[:], AF.Copy,
                                     accum_out=stats[:, h, 0:1])
                junk = pBs.tile([DQ, DK], F32, tag="junk", name="junk")
                nc.scalar.activation(junk[:], sc_ps[:], AF.Square,
                                     accum_out=stats[:, h, 1:2])

            if "phaseB" not in ablate:
                # cross-partition reduce of stats; every partition gets totals
                tot_ps = psB.tile([DQ, H, 2], F32, tag="psb", name="tot_ps")
                nc.tensor.matmul(tot_ps[:], ones_sb[:], stats[:],
                                 start=True, stop=True)
                nc.scalar.mul(mu_all[:], tot_ps[:, :, 0:1], 1.0 / NORM_CNT)
                nc.scalar.mul(m2_all[:], tot_ps[:, :, 1:2], 1.0 / NORM_CNT)
                nc.scalar.square(mu2_all[:], mu_all[:])
                nc.vector.tensor_sub(var_all[:], m2_all[:], mu2_all[:])
                nc.scalar.activation(sd_all[:], var_all[:], AF.Sqrt,
                                     bias=eps_sb[:, 0:1])
                nc.vector.reciprocal(rstd_all[:], sd_all[:])
                # softmax over ek of rstd*scores: the mean shift cancels in
                # softmax, and no max-shift is needed -- scores are z-scored
                # by rstd so |exponent| stays ~<=8, far from fp32 overflow.
                for h in range(H):
                    nc.scalar.activation(e_all[:, h, :], sc_all[:, h, :],
                                         AF.Exp, scale=rstd_all[:, h:h + 1],
                                         accum_out=den_all[:, h:h + 1])
                nc.vector.reciprocal(rec_all[:], den_all[:])
                # fold the softmax denominator into e_all now so the context
                # evacuations in phase C are plain (engine-agnostic) copies
                for h in range(H):
                    eng = nc.vector if h % 2 == 0 else nc.gpsimd
                    eng.tensor_scalar_mul(e_all[:, h, :], e_all[:, h, :],
                                          rec_all[:, h:h + 1])
                # exp.T chunks via XBAR; chunk j=0 holds dk' in [0:128)
                # (112 used), j=1 holds dk' in [112:240)
                for h in range(H):
                    for j in range(KCH):
                        off = 0 if j == 0 else DK - 128
                        nc.sync.dma_start(ptT_sb[:, h, j, :],
                                          e_all[:, h, off:off + 128],
                                          transpose=True)
                # P2.T = (Wv.T-chunks applied to exp.T): [dk-chunk, 128c]
                for h in range(H):
                    for c in range(2):
                        p2w = OV0 if c == 0 else OV1
                        fc0 = 0 if c == 0 else DK - 128
                        p2t_ps = psB.tile([PT, DQ], F32, tag="psb",
                                          name="p2t_ps")
                        nc.tensor.matmul(
                            p2t_ps[0:p2w, :],
                            wvb_sb[0:OV0, 0, fc0:fc0 + p2w],
                            ptT_sb[0:OV0, h, 0, :], start=True, stop=False)
                        nc.tensor.matmul(
                            p2t_ps[0:p2w, :],
                            wvb_sb[0:OV1, 1, fc0:fc0 + p2w],
                            ptT_sb[0:OV1, h, 1, :], start=False, stop=True)
                        nc.scalar.copy(p2t_sb[0:p2w, h, c, :], p2t_ps[0:p2w, :])

        # --- phase C: ctx_h = P2_h @ Kc_h.T, then O1 = sum_h ctx_h.T @ WoT_h --
        # Software-pipelined: the context matmuls for chunk n+1 are emitted
        # before chunk n's output projection, so the PE never stalls on the
        # PSUM evacuations (spread round-robin over DVE / ACT / Pool).
        with (tc.tile_pool(name="psC", bufs=CX_BUFS, space="PSUM")
              if "phaseC" not in ablate else nullcontext()) as psC:
            # GPSIMD/Pool cannot access PSUM, so evacuations alternate DVE/ACT
            evac_i = [0]

            def evac(dst, src):
                i = evac_i[0] % 2
                evac_i[0] += 1
                if i == 0:
                    nc.vector.tensor_copy(dst, src)
                else:
                    nc.scalar.copy(dst, src)

            def emit_ctx(nch):
                ncols = slice(nch * 512, (nch + 1) * 512)
                ctx = pC.tile([DQ, H, 512], BF16, tag="ctx", name="ctx", bufs=3)
                for h in range(H):
                    cx_ps = psC.tile([DQ, 512], F32, tag="cx", name="cx_ps",
                                     bufs=CX_BUFS)
                    nc.tensor.matmul(cx_ps[:], p2t_sb[0:OV0, h, 0, :],
                                     eaT_sb[0:OV0, 2 * h, ncols],
                                     start=True, stop=False)
                    nc.tensor.matmul(cx_ps[:], p2t_sb[0:OV1, h, 1, :],
                                     eaT_sb[0:OV1, 2 * h + 1, ncols],
                                     start=False, stop=True)
                    evac(ctx[:, h, :], cx_ps[:])
                return ctx

            def emit_oproj(nch, ctx):
                for t in range(4):
                    i = nch * 4 + t
                    rows = slice(i * PT, (i + 1) * PT)
                    o_ps = psC.tile([PT, C], F32, tag="o", name="o_ps",
                                    bufs=O_BUFS)
                    for h in range(H):
                        nc.tensor.matmul(o_ps[:],
                                         ctx[:, h, t * PT:(t + 1) * PT],
                                         wotb_sb[:, h, :],
                                         start=(h == 0), stop=(h == H - 1))
                    o_sb = pC.tile([PT, C], F32, tag="o_sb", name="o_sb")
                    evac(o_sb[:], o_ps[:])
                    nc.scalar.dma_start(o1[rows, :], o_sb[:])

            if "phaseC" not in ablate:
                prev = emit_ctx(0)
                for nch in range(1, NCH):
                    cur = emit_ctx(nch)
                    emit_oproj(nch - 1, prev)
                    prev = cur
                emit_oproj(NCH - 1, prev)

        if dbg_t is not None:
            for name, src in (("d_s", s_sb), ("d_e", e_all),
                              ("d_p2t", p2t_sb), ("d_eaT", eaT_sb)):
                nc.sync.dma_start(dbg_t[name][:], src[:])


_NC_CACHE = None


def get_nc():
    global _NC_CACHE
    if _NC_CACHE is None:
        _NC_CACHE = build_nc()
    return _NC_CACHE


def make_in_maps(emb1, emb_all, Wq, Wk, Wv, Wo):
    emb1 = np.ascontiguousarray(np.asarray(emb1, dtype=np.float32))
    emb_all = np.ascontiguousarray(np.asarray(emb_all, dtype=np.float32))
    Wq = np.asarray(Wq, dtype=np.float32)
    Wk = np.asarray(Wk, dtype=np.float32)
    Wv = np.asarray(Wv, dtype=np.float32)
    Wo = np.asarray(Wo, dtype=np.float32)

    scale = 1.0 / np.sqrt(np.float32(KV))
    wqt_np = np.ascontiguousarray(np.transpose(Wq, (2, 0, 1)) * scale)  # [c,h,e]
    wkt_full = Wk.T                                                     # [k,ek]
    wkt_np = np.ascontiguousarray(
        wkt_full.reshape(KCH, KHW, DK).transpose(1, 0, 2))              # [120,2,240]
    # Wv rows in the two overlapping dk' chunks: [0:128) and [112:240)
    wvb_np = np.stack([Wv[0:128, :], Wv[DK - 128:DK, :]], axis=1)       # [128,2,240]
    wvb_np = np.ascontiguousarray(wvb_np).astype(ml_dtypes.bfloat16)
    wotb_np = np.ascontiguousarray(
        Wo.reshape(C, DQ, H).transpose(1, 2, 0)).astype(ml_dtypes.bfloat16)

    shared = {"wqt": wqt_np, "wkt": wkt_np, "wvb": wvb_np, "wotb": wotb_np}
    return [
        {"e1": emb1[b], "ea": emb_all[b], **shared}
        for b in range(B)
    ]


def run(inputs, trace=False, **spmd_kwargs):
    nc = get_nc()
    in_maps = make_in_maps(**inputs)
    res = run_bass_kernel_spmd(nc, in_maps, list(range(B)), trace=trace,
                               **spmd_kwargs)
    out = np.stack([np.asarray(res.results[b]["o1"]) for b in range(B)], axis=0)
    return out.astype(np.float32, copy=False), res


def kernel(**inputs) -> np.ndarray:
    out, _ = run(inputs, trace=False)
    return out
